# revision 66
# baseline (speedup 1.0000x reference)
"""Trainium2 Bass kernel for nn_BioRNN: 1000-step leaky-relu RNN scan.

Math per step (reference):
    r_t = relu(h_t)
    y_t = r_t @ W_out_w.T + W_out_b
    h_{t+1} = (1-DT) h_t + DT (x_t @ W_in.T + r_t @ W_rec.T + bias)

Device mapping (per core, batch-sharded 4096 -> 8 x 512):
  - State kept feature-major [H, B] in SBUF as +/- relu parts so ONE
    activation per step per batch-chain produces everything the next step
    needs (relu(h), relu(-h)) plus +/- y staging rows; y = y+ - y- is
    reconstructed on the host.
  - One constant 121x112 matmul per step per chain computes
    [h', -h', y', -y'] from rhs rows [s+(50); s-(50); y+-(12, zero weight);
    x(8); ones(1)].  Decay, input proj, recurrent proj and both biases are
    folded into the weight matrix (ones-row supplies the biases).
  - Everything on the PE/DMA path is fp16 (state, weights, x, y): fp16
    matmuls stream at 1 cycle/row vs fp32's 4, and fp16 state keeps the
    1000-step accumulated error ~2e-3 (bf16 would be ~2e-2: too much).
    PSUM accumulation stays fp32.
  - The step loop is LATENCY-bound, not engine-bound: per-chain cycle =
    relu exec + ~450ns fixed (relu write-ack, two sem hops, PE pipeline
    drain).  4 chains of 128 cols; each chain ALTERNATES its relu between
    ACT and DVE by step parity.  Alternation (a) averages the two engines'
    leg latencies (ACT leg 404+0.83a ns, DVE leg 285+1.04a ns) and (b)
    locks the engines into step-parity lockstep through the data deps,
    killing the multi-us convoy stalls that static engine assignment
    suffers at every perturbation (the in-order PE with a 4-deep wait
    queue is the coupling point).
  - rhs slots live in a ring at fixed stride; X loads / Y stores move CHUNK
    steps per DMA.  CHUNK=16 amortizes the ~1us SWDGE descriptor-gen cost
    per DMA on the Pool engine (3 DMAs/chunk).  The weight block occupies
    ring cols 0..111; one init DMA covers weights + the full slot 0 so the
    first matmul waits on a single DMA.
  - Ring-cycle WAW deps (NBUF even -> same engine) and stale writer deps
    on partially-overwritten rectangles are try_remove'd so every relu and
    DMA trigger keeps a single sem wait (walrus 1-wait ISA limit).
"""

import os

import numpy as np

import concourse.bass as bass
import concourse.tile as tile
from concourse import mybir
from concourse.bass_utils import run_bass_kernel_spmd
from concourse.tile import add_dep_helper

DT = 0.1
H, IN, OUT = 50, 8, 6
T, B = 1000, 4096
NCORES = 8
BC = B // NCORES  # 512 batch columns per core

# rhs ring row layout. Compute-engine APs need a 32-aligned partition base, so
# the relu-written block [s+, s-, y+, y-] sits at rows 0..111 and the
# DMA-written block [x, ones] at rows 112..120. The matmul contracts over all
# 121 rows with zero weights on the y rows.
RSP = 0     # relu(h) rows 0..49
RSN = 50    # relu(-h) rows 50..99
RY = 100    # y+/- rows 100..111
RX = 112    # x rows 112..119
RONE = 120  # ones row
NROWS = 121
K = 121     # matmul contraction rows (s+, s-, [y ignored], x, ones)
M = 112     # matmul output rows (h', -h', y', -y')
W0 = 112    # weight block cols 0..111; ring slot columns start here

PSUM_BUFS = 1         # psum slots per chain (the matmul already waits for the
                      # relu via the rhs RAW, same condition as the PSUM WAR;
                      # bufs=2 measures identical, and 1 frees a bank for the
                      # PSUM carrier scratch)
CHUNK = 16            # steps per DMA chunk (large: amortize SWDGE gen cost)
NSEC = 6              # ring sections
NBUF = NSEC * CHUNK   # ring slots

F16 = mybir.dt.float16
F32 = mybir.dt.float32

# batch chains: (col_start, ncols). Each chain's relu ALTERNATES between the
# ACT and DVE engines by step parity (engine = act iff (ci+t) even). The
# per-step floor here is per-chain round-trip LATENCY (relu exec + ~450ns of
# ack/sem/PE-drain), not engine work; alternation (a) averages the two
# engines' leg latencies and (b) locks the engines into step-parity lockstep
# through the data deps themselves, killing the PE head-of-line convoy that
# static engine assignment suffers at every chunk boundary.
# (col_start, ncols, mode): mode "alt0"/"alt1" alternate ACT/DVE by step
# parity (opposite phases), "vec" stays on DVE every step
CHAINS = (
    (0, 128, "alt0"),
    (128, 128, "alt0"),
    (256, 128, "alt1"),
    (384, 128, "alt1"),
)
assert sum(c[1] for c in CHAINS) == BC
# chains grouped into shared PSUM tiles (disjoint col ranges in one bank).
# One chain per group: sharing a tile couples the member chains through the
# pool-rotation WAR (measured ~35% slower).
PSGROUPS = ((0,), (1,), (2,), (3,))


def chain_engine(mode: str, t: int) -> str:
    if mode == "vec":
        return "vec"
    ph = 0 if mode == "alt0" else 1
    return "act" if (t + ph) % 2 == 0 else "vec"


def _build_G(W_in, W_rec, bias, W_out_w, W_out_b):
    G = np.zeros((M, K), np.float32)
    G[0:50, RSP : RSP + 50] = (1.0 - DT) * np.eye(50, dtype=np.float32) + DT * W_rec
    G[0:50, RSN : RSN + 50] = -(1.0 - DT) * np.eye(50, dtype=np.float32)
    G[0:50, RX : RX + 8] = DT * W_in
    G[0:50, RONE] = DT * bias
    G[50:100] = -G[0:50]
    G[100:106, RSP : RSP + 50] = W_out_w
    G[100:106, RONE] = W_out_b
    G[106:112] = -G[100:106]
    return np.ascontiguousarray(G.T.astype(np.float16))  # lhsT [K, M]


def _build_bass(t_steps: int):
    """Build the SPMD Bass program for t_steps."""
    nchx = (t_steps + CHUNK - 1) // CHUNK
    nchy = ((t_steps + 1) + CHUNK - 1) // CHUNK
    nc = bass.Bass("TRN2", debug=False, enable_asserts=False, num_devices=NCORES)
    x_d = nc.dram_tensor("x", [nchx, IN + 1, CHUNK * BC], F16, kind="ExternalInput").ap()
    # init carries weights + zero state for SLOT 0 only: slots 1.. are always
    # relu-written before they are read, so zeroing the whole first chunk
    # would only add ~4us of 100-partition DMA to the critical startup path
    init_d = nc.dram_tensor(
        "init", [NROWS, W0 + BC], F16, kind="ExternalInput"
    ).ap()
    wb16_d = nc.dram_tensor("wb16", [1, 2], mybir.dt.bfloat16, kind="ExternalInput").ap()
    y_d = nc.dram_tensor("y", [nchy, 12, CHUNK, BC], F16, kind="ExternalOutput").ap()

    with tile.TileContext(nc) as tc:
        with tc.tile_pool(name="ring", bufs=1) as rpool, tc.tile_pool(
            name="psum", bufs=PSUM_BUFS, space="PSUM"
        ) as ppool:
            ring = rpool.tile([NROWS, W0 + NBUF * BC], F16)
            scr = rpool.tile([1, 4096], F32, tag="scr")
            scrw = rpool.tile([1, 2], mybir.dt.bfloat16, tag="scrw")
            scr_idx = [0]
            w_s = ring[0:K, 0:M]

            # ONE init DMA covers weights + the full slot 0 (zero state, zero
            # y rows, x(0) + ones): the first matmul then waits a single DMA
            # chain (~2.5us) instead of three serialized HWDGE dispatches.
            init_i = nc.sync.dma_start(out=ring[:, 0 : W0 + BC], in_=init_d)
            scrw_i = nc.sync.dma_start(out=scrw[:, :], in_=wb16_d)
            gp_dmas = []

            # Steady-state X/Y DMAs go through SWDGE (gpsimd): their trigger
            # instructions live in the POOL engine stream, so POOL carriers
            # can absorb data-dep ticks and each trigger keeps <=1 wait. The
            # few init DMAs stay on HWDGE (<=8, so no lane reuse / no queue
            # waits).
            def xdma(c, first=False):
                sec = W0 + (c % NSEC) * CHUNK * BC
                eng = nc.sync if first else nc.gpsimd
                if c == 0:
                    # slot 0's x rows ride in the init DMA; chunk 0's x DMA
                    # covers slots 1..CHUNK-1 only (no overlap with init)
                    d = eng.dma_start(
                        out=ring[RX:NROWS, sec + BC : sec + CHUNK * BC],
                        in_=x_d[0][:, BC:],
                    )
                else:
                    d = eng.dma_start(
                        out=ring[RX:NROWS, sec : sec + CHUNK * BC], in_=x_d[c]
                    )
                if not first:
                    gp_dmas.append(d)
                # WAW vs ALL of the section's previous x writers is already
                # implied transitively by the WAR on the matmuls that read
                # them (the region tracker conservatively keeps stale writers
                # of partially-overwritten rectangles, e.g. slot 0)
                for c2 in range(c % NSEC, c, NSEC):
                    if c2 in xdmas:
                        d.ins.try_remove_dependency(xdmas[c2].ins.name)
                if c % NSEC == 0 and c > 0:
                    # init owns slot 0's x rows on the first pass
                    d.ins.try_remove_dependency(init_i.ins.name)
                return d

            def ydma(c, p0=None, p1=None):
                # one DMA per chunk (optionally a position sub-range [p0,p1)
                # of the chunk, used to split the final chunk); its producing
                # relus span both engines (alternation), so pool carriers
                # absorb both engine ticks before the trigger. chunk 0 skips
                # slot 0 (y rows unwritten, position 0 garbage)
                sec = W0 + (c % NSEC) * CHUNK * BC
                s0 = 1 if c == 0 else 0
                if p0 is not None:
                    s0 = max(s0, p0)
                s1 = CHUNK if p1 is None else p1
                src = ring[RY : RY + 12, sec + s0 * BC : sec + s1 * BC]
                src = src.rearrange("p (s b) -> p s b", b=BC)
                d = nc.gpsimd.dma_start(out=y_d[c][:, s0:s1, :], in_=src)
                if c % NSEC == 0 and c > 0:
                    # stale RAW on init (slot 0's zeroed y rows were long
                    # since relu-overwritten; that relu RAW is the real dep
                    # and is absorbed by the pool carriers)
                    d.ins.try_remove_dependency(init_i.ins.name)
                gp_dmas.append(d)
                return d

            # Each ISA instruction has ONE sem-wait slot (Matmult two via its
            # LDWEIGHTS). Tile emits a wait for every dep whose tick the
            # engine hasn't observed, so any op with >1 foreign producer
            # over-fills the slot. Wait-carriers fix this: tiny same-engine
            # ops that sync-depend on a DMA, advancing the engine's observed
            # tick so the real relu needs only its matmul wait.
            prev_carrier = {"act": None, "vec": None, "pe": None, "pool": None}

            def carrier(eng, deps):
                # tiny same-engine op that sync-depends on `deps`, advancing
                # the engine's observed ticks so the next real op needs only
                # its single architectural wait slot
                if eng == "pe":
                    # standalone LDWEIGHTS (bf16, tiny); the next real matmul
                    # reloads its own weights
                    c = nc.tensor.ldweights(scrw[0:1, 0:1])
                elif eng == "pool":
                    i = scr_idx[0]
                    scr_idx[0] += 1
                    assert i < 4096
                    c = nc.gpsimd.memset(scr[0:1, i : i + 1], 0.0)
                elif eng == "act":
                    i = scr_idx[0]
                    scr_idx[0] += 1
                    assert i < 4096
                    c = nc.scalar.activation(
                        scr[0:1, i : i + 1],
                        scrw[0:1, 0:1],
                        mybir.ActivationFunctionType.Copy,
                        bias=0.0,
                    )
                else:
                    i = scr_idx[0]
                    scr_idx[0] += 1
                    assert i < 4096
                    c = nc.vector.tensor_copy(scr[0:1, i : i + 1], scrw[0:1, 0:1])
                for d in deps:
                    add_dep_helper(c.ins, d.ins, sync=True, reason="wait-carrier")
                if prev_carrier[eng] is not None:
                    add_dep_helper(
                        c.ins, prev_carrier[eng].ins, sync=False, reason="order"
                    )
                prev_carrier[eng] = c
                return c

            def pe_carrier(dep):
                return carrier("pe", [dep])

            xdmas, ydmas = {}, {}
            xdmas[0] = xdma(0, first=True)
            if nchx > 1:
                xdmas[1] = xdma(1, first=True)

            # pre-loop carrier chains: first call absorbs the scrw-DMA tick,
            # the second the init lane. First matmul reads init (weights +
            # slot 0); the chunk-5 flip relu later WAW-overwrites slot 0.
            carrier("act", [])
            ca = carrier("act", [init_i])
            carrier("vec", [])
            cv = carrier("vec", [init_i])
            carrier("pe", [])
            carrier("pe", [init_i])
            last_relu = {"act": None, "vec": None}
            last_mm = None
            chunk_last = {}
            slot_writer = {}  # (ring slot, chain) -> relu that last wrote it
            ydma_a = None

            def after_pool_carrier(d):
                add_dep_helper(
                    d.ins, prev_carrier["pool"].ins, sync=False, reason="order"
                )

            for t in range(t_steps):
                if t % CHUNK == 0:
                    k = t // CHUNK
                    if k >= 1:
                        chunk_last[k - 1] = dict(last_relu)
                    if k >= 2:
                        # y DMA lags one extra chunk (k-2, not k-1) for slack;
                        # pool carriers absorb both engines' last-relu ticks
                        # so the single trigger has no unobserved waits. At
                        # the final boundary also flush k-1 so the end-of-
                        # program tail has only one y DMA left to issue.
                        if k < nchx - 2:
                            flush = [k - 2]
                        elif k == nchx - 2:
                            # start draining early: pull k-1 forward so the
                            # last boundary has a single flush and the end-of-
                            # program pool gen never stacks two ydmas
                            flush = [k - 2, k - 1]
                        else:
                            flush = [k - 1]
                        for c in flush:
                            carrier("pool", [chunk_last[c]["act"]])
                            carrier("pool", [chunk_last[c]["vec"]])
                            yd = ydma(c)
                            after_pool_carrier(yd)
                            ydmas[c] = yd
                    if k + 2 < nchx:
                        carrier("pool", [last_mm] if last_mm is not None else [])
                        xdmas[k + 2] = xdma(k + 2)
                        after_pool_carrier(xdmas[k + 2])
                    if k in xdmas:
                        # absorb this chunk's x-DMA lane tick before the
                        # first matmul that reads the fresh x rows
                        pe_carrier(xdmas[k])
                    if k >= 3:
                        # chunk k's relus overwrite the sections ydma(k-6)/
                        # ydma(k-5) read (previous ring pass); absorb the
                        # newest safe ydma tick in both relu engines first
                        ca = carrier("act", [ydmas[k - 3]])
                        cv = carrier("vec", [ydmas[k - 3]])
                    # (ring-cycle WAW handled per-relu: NBUF is even, so the
                    # overwriting relu is always on the SAME engine as the one
                    # it overwrites -> program order enforces the WAW and the
                    # dep is try_remove'd below; no absorb carrier needed)
                if t == t_steps - 1:
                    # flush the final chunk's y slots that only need relus
                    # through step t-1: the ~1us SWDGE descriptor-gen then
                    # overlaps the last step instead of serializing after it
                    cl = nchy - 1
                    pl = t_steps - cl * CHUNK  # final chunk position of the
                    if 0 < pl:                 # last y slot (= slot t_steps)
                        carrier("pool", [last_relu["act"]])
                        carrier("pool", [last_relu["vec"]])
                        ydma_a = ydma(cl, 0, pl)
                        after_pool_carrier(ydma_a)
                sr = W0 + (t % NBUF) * BC        # slot base this step reads
                sw = W0 + ((t + 1) % NBUF) * BC  # slot base the relu writes
                # one PSUM tile per group; each member chain gets a disjoint
                # column slice of it (all within one 2KB bank)
                gtiles = {}
                for gi, grp in enumerate(PSGROUPS):
                    gcols = sum(CHAINS[ci][1] for ci in grp)
                    gtiles[gi] = ppool.tile(
                        [M, gcols], F32, tag=f"ps{gi}", name=f"ps{gi}"
                    )
                for ci, (c0, cn, mode) in enumerate(CHAINS):
                    eng = chain_engine(mode, t)
                    for gi, grp in enumerate(PSGROUPS):
                        if ci in grp:
                            off = sum(CHAINS[cj][1] for cj in grp if cj < ci)
                            ps = gtiles[gi][:, off : off + cn]
                            break
                    mm = nc.tensor.matmul(
                        ps,
                        w_s,
                        ring[0:K, sr + c0 : sr + c0 + cn],
                        start=True,
                        stop=True,
                    )
                    if ci == 0 and prev_carrier["pe"] is not None:
                        add_dep_helper(
                            mm.ins, prev_carrier["pe"].ins, sync=False, reason="order"
                        )
                    last_mm = mm
                    dst = ring[0:M, sw + c0 : sw + c0 + cn]
                    if eng == "act":
                        r = nc.scalar.activation(
                            dst, ps, mybir.ActivationFunctionType.Relu
                        )
                        if ca is not None:
                            add_dep_helper(r.ins, ca.ins, sync=False, reason="order")
                    else:
                        r = nc.vector.tensor_scalar_max(dst, ps, 0.0)
                        if cv is not None:
                            add_dep_helper(r.ins, cv.ins, sync=False, reason="order")
                    last_relu[eng] = r
                    # ring-cycle WAW vs the same chain's relu NBUF steps ago:
                    # NBUF is even so it is the SAME engine -> program order
                    # already enforces it; drop the dep so the relu keeps a
                    # single sem wait (its matmul)
                    slot = (t + 1) % NBUF
                    old = slot_writer.get((slot, ci))
                    if old is not None:
                        r.ins.try_remove_dependency(old.ins.name)
                    if slot == 0:
                        # slot 0's first-pass writer is the init DMA (stale
                        # after the first overwrite); transitively enforced
                        # through the chain's own mm->relu history
                        r.ins.try_remove_dependency(init_i.ins.name)
                    slot_writer[(slot, ci)] = r
            # final y slot (y of the last step) via HWDGE on SP: the slot's
            # producers split cleanly by engine (the last step's chains 0,1
            # vs 2,3 are on opposite engines), so two half-width DMAs each
            # carry a SINGLE engine-sem wait. HWDGE lanes used: init, scrw,
            # x0, x1 + these 2 = 6 <= 8, no lane reuse.
            cl = nchy - 1
            pl = t_steps - cl * CHUNK
            secl = W0 + (cl % NSEC) * CHUNK * BC
            srcl = ring[RY : RY + 12, secl + pl * BC : secl + (pl + 1) * BC]
            half = CHAINS[0][1] + CHAINS[1][1]
            yb = []
            for lo, hi in ((0, half), (half, BC)):
                d = nc.sync.dma_start(
                    out=y_d[cl][:, pl : pl + 1, lo:hi],
                    in_=srcl.rearrange("p (s b) -> p s b", b=BC)[:, :, lo:hi],
                )
                if yb:
                    add_dep_helper(
                        d.ins, yb[-1].ins, sync=False, reason="order"
                    )
                yb.append(d)
            ydma_b = yb[-1]
            # any remaining un-issued full chunks (tiny t_steps edge case)
            carrier("pool", [last_relu["act"]])
            carrier("pool", [last_relu["vec"]])
            for c in range(nchy):
                if c not in ydmas and c != cl:
                    yd = ydma(c)
                    after_pool_carrier(yd)
                    ydmas[c] = yd

            # SP-nop chain: one dep each, so the TileContext tail drain (an SP
            # instruction waiting for every proc's final tick) finds all its
            # ticks already observed and stays within its single wait slot
            sinks = [scrw_i, init_i, xdmas[0]]
            sinks += yb
            if 1 in xdmas:
                sinks.append(xdmas[1])
            sinks += gp_dmas[-8:]
            sinks += [last_mm, last_relu["act"], last_relu["vec"]]
            if prev_carrier["pool"] is not None:
                sinks.append(prev_carrier["pool"])
            prev_nop = None
            for s in sinks:
                n = nc.sync.nop()
                add_dep_helper(n.ins, s.ins, sync=True, reason="drain-prewait")
                if prev_nop is not None:
                    add_dep_helper(n.ins, prev_nop.ins, sync=False, reason="order")
                prev_nop = n
    return nc


def _prep_x(input_core: np.ndarray, t_steps: int):
    """(T, BC, IN) fp32 -> chunked (nchx, IN+1, CHUNK*BC) fp16 contiguous;
    feature row IN is the constant-ones row used for the folded biases."""
    nchx = (t_steps + CHUNK - 1) // CHUNK
    xt = np.zeros((nchx * CHUNK, IN + 1, BC), np.float16)
    xt[:t_steps, :IN] = input_core.transpose(0, 2, 1).astype(np.float16)
    xt[:, IN] = 1.0
    xc = xt.reshape(nchx, CHUNK, IN + 1, BC).transpose(0, 2, 1, 3)
    return np.ascontiguousarray(xc.reshape(nchx, IN + 1, CHUNK * BC))


def _prep_init(lhsT: np.ndarray, x_chunk0: np.ndarray):
    """[K, M] fp16 weights + first x chunk -> [NROWS, W0+BC]: weights block,
    zeroed slot-0 state/y rows, and slot 0's x/ones rows."""
    init = np.zeros((NROWS, W0 + BC), np.float16)
    init[:, 0:W0] = lhsT
    init[RX:NROWS, W0:] = x_chunk0[:, 0:BC]
    return np.ascontiguousarray(init)


def kernel(input_seq, W_in, W_rec, bias, W_out_w, W_out_b):
    input_seq = np.asarray(input_seq, dtype=np.float32)
    lhsT = _build_G(
        np.asarray(W_in, np.float32),
        np.asarray(W_rec, np.float32),
        np.asarray(bias, np.float32),
        np.asarray(W_out_w, np.float32),
        np.asarray(W_out_b, np.float32),
    )
    t_steps = input_seq.shape[0]
    nc = _build_bass(t_steps)
    import ml_dtypes

    wb16 = np.zeros((1, 2), dtype=ml_dtypes.bfloat16)
    in_maps = []
    for c in range(NCORES):
        xc = _prep_x(input_seq[:, c * BC : (c + 1) * BC, :], t_steps)
        in_maps.append({"x": xc, "init": _prep_init(lhsT, xc[0]), "wb16": wb16})
    trace = bool(int(os.environ.get("KERNEL_TRACE", "0")))
    res = None
    last_exc = None
    # Tile scheduling has run-to-run nondeterminism; on a rare bad roll the
    # sem assignment can exceed the 1-wait ISA slot and walrus rejects the
    # build. A fresh rebuild re-rolls the schedule, so retry.
    for attempt in range(3):
        try:
            res = run_bass_kernel_spmd(
                nc, in_maps, core_ids=list(range(NCORES)), trace=trace and attempt == 0
            )
            break
        except (ImportError, ModuleNotFoundError):
            trace = False
            continue
        except Exception as e:  # compile/schedule failure — rebuild and retry
            last_exc = e
            nc = _build_bass(t_steps)
    if res is None:
        raise last_exc
    kernel.last_results = res

    nchy = ((t_steps + 1) + CHUNK - 1) // CHUNK
    outs = []
    for c in range(NCORES):
        y = res.results[c]["y"]
        yd = y[:, 0:6].astype(np.float32) - y[:, 6:12].astype(np.float32)
        yd = yd.transpose(0, 2, 3, 1).reshape(nchy * CHUNK, BC, OUT)
        outs.append(yd[1 : t_steps + 1])           # position t+1 holds y_t
    return np.ascontiguousarray(np.concatenate(outs, axis=1))


kernel.last_results = None
